# revision 54
# baseline (speedup 1.0000x reference)
"""Trainium2 Bass kernel for a 3-branch GCN layer (sum of three GCNConvs).

Math: out[b,t] = sum_k A_k @ (x[b,t] @ W_k) + b_k over a tiny shared
25-node graph. Equivalently, per output node n:
    out[:, n, :] = sum_{m in S_n} x[:, m, :] @ B_{m,n},
    B_{m,n} = sum_k A_k[n, m] * W_k            (64x64 blocks)
where S_n is the set of source nodes with any edge into n (incl. self
loops). For this graph only ~189 of 625 blocks are nonzero. The graph
(edge_index) is known when kernel() runs, so the Bass program is
compiled per-graph with the block schedule hardcoded.

Device strategy (data-parallel over batch across 8 cores; 2400 rows
per core split into two 1200-row slabs on the SBUF partition halves):

- The PE sequencer processes (LDWEIGHTS|MATMUL) instructions at a
  shared ~23.5ns each, so the PE window is
  max(compute_wall, 23.5ns * n_instr): MINIMIZE INSTRUCTIONS, and
  keep consecutive matmuls of a stream on the SAME psum bank
  (block-major chains that cycle banks per instruction measured ~2.5x
  worse per-instruction spacing). Chains are chain-major: per (node,
  round) an accumulation chain over the node's source blocks, rounds
  of R=400 columns (3 rounds cover the 1200-row slab).
- Weights ride as fp8 e3m4 (x wscale, a pow2 folded into x on host):
  the TRN2 PE supports mixed fp8e3 lhsT x fp16 rhs matmuls bit-exactly
  (verified on HW), halving weight DMA bytes and the weight-arrival
  head gate. End-to-end max-rel error ~1.3e-2 (gate 2e-2).
- 64x64 PE tiling gives 4 concurrent streams: T0/T2 read SBUF
  partitions 0-63 (slab A), T8/T10 read 64-127 (slab B); T0/T8 chain
  node na of each pair, T2/T10 node nb.
- PSUM: all 8 banks as an explicit rotating ring; step j uses banks
  (6j+k)%8, k=0..5 (r0/r1/r2 for each slab side), giving ~1.3 steps of
  evacuation slack.
- x rides as three per-round segments; the first weight chunk, then
  the whole round-0 segment, then the remaining weights lead the sync
  queue (the scalar HWDGE queue runs ~4x slower whenever the sync
  queue is active). The PE p-state throttle resets on stalls, so the
  PE intentionally starts only when round 0 is resident (~18us) and
  then runs gapless at the ~46.5ns/instruction sequencer floor.
- PSUM banks evacuate [128, R] fp32->fp16 (vector/scalar engines
  alternating) into per-round staging, flushed in a few large DMAs;
  the final round flushes in four chunks so the tail flush is ~200KB.
"""

import sys

import numpy as np

if "/opt/trn_rl_repo" not in sys.path:
    sys.path.insert(0, "/opt/trn_rl_repo")

B, T, NNODES, C = 64, 300, 25, 64
N_CORES = 8
ROWS_LOC = (B // N_CORES) * T    # 2400
RSLAB = ROWS_LOC // 2            # 1200 rows per partition half
R = 400                          # rows per round (chain rhs width)
NROUNDS = RSLAB // R             # 3
NPAIR = (NNODES + 1) // 2        # 13
# per-round staging layout: round block = NPAIR steps x (A slot | B slot)
RNDCOLS = NPAIR * 2 * R          # 10400
XSEG = NNODES * R                # x cols per round segment (10000)
# flush boundaries (after step j of round r -> steps [f0, j] flushed);
# the last round flushes in five chunks so the final DMAs are small
# and wait on as little evacuation as possible (psum-direct DMA of the
# last step is NOT possible: dma_start rejects PSUM sources)
FLUSHES = {0: (6, 12), 1: (6, 12), 2: (4, 7, 10, 11, 12)}
# weight DMA chunks in pair units (pairs [a, b))
W_CHUNKS = ((0, 1), (1, 6), (6, NPAIR))

_PROGRAM_CACHE = {}
# extra kwargs for run_bass_kernel_spmd (test harness sets trace=True here)
_RUN_KW = {}
# Ship weights as fp8e3 on the wire (halves W DMA bytes and the round-0
# arrival gate) and convert once on-device to fp16 on the then-idle
# vector/scalar engines: mixed fp8e3xfp16 matmuls measured ~+7.5ns per
# matmul slot (+10us end to end), so the PE must see fp16 weights.
W_WIRE_FP8 = True


def _dense_adj(edge_index_k: np.ndarray) -> np.ndarray:
    """PyG GCNConv normalized dense adjacency A[dst, src] (float64)."""
    row = edge_index_k[0].astype(np.int64)
    col = edge_index_k[1].astype(np.int64)
    loop = np.arange(NNODES, dtype=np.int64)
    row = np.concatenate([row, loop])
    col = np.concatenate([col, loop])
    deg = np.zeros(NNODES, dtype=np.float64)
    np.add.at(deg, col, 1.0)
    dinv = np.where(deg > 0, 1.0 / np.sqrt(deg), 0.0)
    norm = dinv[row] * dinv[col]
    A = np.zeros((NNODES, NNODES), dtype=np.float64)
    np.add.at(A, (col, row), norm)
    return A


def _plan(edge_index, Ws):
    """Block schedule from the actual graph.

    order:   nodes sorted by descending source count
    src[n]:  source nodes of output node n
    w8:      [64, TOT*64] fp8e3 packed B_{m,n}*wscale blocks (node-major
             in `order`, sources in src[n] order)
    off[n]:  first block index of node n in w8
    pairs:   [(na, nb)] lockstep chain pairs (nb None on the odd tail)
    fu:      nodes in first-use order (order x columns are packed/DMA'd)
    pos[m]:  position of node m in fu
    """
    A = [_dense_adj(edge_index[k]) for k in range(3)]
    src = []
    for n in range(NNODES):
        s = [m for m in range(NNODES) if any(Ak[n, m] != 0.0 for Ak in A)]
        src.append(s)
    order = sorted(range(NNODES), key=lambda n: -len(src[n]))
    tot = sum(len(s) for s in src)
    wblocks = np.zeros((64, tot * 64), dtype=np.float64)
    off = {}
    idx = 0
    for n in order:
        off[n] = idx
        for m in src[n]:
            Bmn = sum(A[k][n, m] * Ws[k] for k in range(3))  # [c_in, c_out]
            wblocks[:, idx * 64:(idx + 1) * 64] = Bmn
            idx += 1

    if W_WIRE_FP8:
        # largest pow2 scale keeping wscale*B inside e3m4 range (15.5)
        absmax = np.abs(wblocks).max()
        wscale = float(2.0 ** min(4, int(np.floor(np.log2(15.5 / absmax)))))
        import ml_dtypes

        w8 = (wblocks * wscale).astype(np.float32).astype(
            ml_dtypes.float8_e3m4
        )
    else:
        wscale = 1.0
        w8 = wblocks.astype(np.float16)

    pairs = [
        (order[2 * j], order[2 * j + 1] if 2 * j + 1 < NNODES else None)
        for j in range(NPAIR)
    ]

    # first-use order of source nodes given the emission order
    fu = []
    seen = set()
    for na, nb in pairs:
        sa = src[na]
        sb = src[nb] if nb is not None else []
        for i in range(max(len(sa), len(sb))):
            for s in (sa, sb):
                if i < len(s) and s[i] not in seen:
                    seen.add(s[i])
                    fu.append(s[i])
    assert len(fu) == NNODES
    pos = {m: i for i, m in enumerate(fu)}
    return dict(order=order, src=src, w8=w8, off=off, tot=tot,
                pairs=pairs, fu=fu, pos=pos, wscale=wscale)


def _strip_redundant_ldweights(nc):
    """Drop legalization-inserted LDWEIGHTS whose stationary (same AP,
    same PE tile) is already loaded by the previous LDWEIGHTS for that
    tile. Stationary state is per tile_position; only LDWEIGHTS/matmul
    touch it, and sync is via separate semaphore instructions, so this
    is order-safe."""
    from concourse import mybir

    dropped = 0
    for fn in nc.m.functions:
        for blk in fn.blocks:
            insts = blk.instructions
            out = []
            last = {}
            changed = False
            for inst in insts:
                if isinstance(inst, mybir.InstLdweights):
                    ap = inst.ins[0]
                    tp = str(getattr(inst, "tile_position", None))
                    key = (
                        str(ap.ap), ap.offset, str(ap.dtype),
                        getattr(ap, "memref", None),
                        str(getattr(inst, "perf_mode", None)),
                        str(getattr(inst, "is_transpose", None)),
                        str(getattr(inst, "tile_size", None)),
                    )
                    if last.get(tp) == key:
                        dropped += 1
                        changed = True
                        continue
                    last[tp] = key
                out.append(inst)
            if changed:
                blk.instructions = out
    return dropped


def _build_program(plan):
    import concourse.tile as tile
    from concourse import bacc, mybir

    f32 = mybir.dt.float32
    f16 = mybir.dt.float16
    f8e3 = mybir.dt.float8e3

    src, off, tot = plan["src"], plan["off"], plan["tot"]
    pairs, fu, pos = plan["pairs"], plan["fu"], plan["pos"]
    order = plan["order"]

    xcols = NROUNDS * XSEG
    ocols = NROUNDS * RNDCOLS

    nc = bacc.Bacc(
        "TRN2", target_bir_lowering=False, debug=False, num_devices=N_CORES
    )
    wdt = f8e3 if W_WIRE_FP8 else f16
    xin = nc.dram_tensor("xin", [128, xcols], f16, kind="ExternalInput").ap()
    # weights pre-duplicated on host to both partition halves
    wdev = nc.dram_tensor("wdev", [128, tot * 64], wdt, kind="ExternalInput").ap()
    outd = nc.dram_tensor("outd", [128, ocols], f16, kind="ExternalOutput").ap()

    with tile.TileContext(nc) as tc:
        with (
            tc.tile_pool(name="w", bufs=1) as wpool,
            tc.tile_pool(name="x", bufs=1) as xpool,
            tc.tile_pool(name="o", bufs=1) as opool,
            tc.tile_pool(name="ps", bufs=1, space="PSUM") as pspool,
        ):
            wt = wpool.tile([128, tot * 64], f16, tag="w")
            if W_WIRE_FP8:
                wt8 = wpool.tile([128, tot * 64], f8e3, tag="w8")
            xt = xpool.tile([128, xcols], f16, tag="x")
            ot = opool.tile([128, ocols], f16, tag="o")
            pb = [
                pspool.tile([128, 512], f32, tag=f"pb{i}", name=f"pb{i}")
                for i in range(8)
            ]

            def load_x(rnd, n0, n1):
                """Round-rnd segment x cols of first-use positions [n0, n1)."""
                c0, c1 = rnd * XSEG + n0 * R, rnd * XSEG + n1 * R
                nc.sync.dma_start(xt[:, c0:c1], xin[:, c0:c1])

            def load_w(p0, p1):
                """Blocks of pairs [p0, p1) (contiguous in order); fp8
                wire chunks convert to fp16 on DVE+ACT (idle early)."""
                b0 = off[order[2 * p0]] * 64
                b1 = off[order[2 * p1]] * 64 if 2 * p1 < NNODES else tot * 64
                if not W_WIRE_FP8:
                    nc.sync.dma_start(wt[:, b0:b1], wdev[:, b0:b1])
                    return
                nc.sync.dma_start(wt8[:, b0:b1], wdev[:, b0:b1])
                mid = (b0 + b1) // 2
                nc.vector.tensor_copy(wt[:, b0:mid], wt8[:, b0:mid])
                nc.scalar.copy(wt[:, mid:b1], wt8[:, mid:b1])

            # priority order on the single sync queue. The PE p-state
            # throttles (0.65/1.2GHz) reset on every stall, so a
            # stall-pocked early stream costs far more than a later
            # gapless start: load the first weight chunk (so its fp16
            # conversion hides under the x0 transfer), then ALL of the
            # round-0 x segment in one transfer, then the remaining
            # weights; the PE starts ~18us in and never stalls again.
            load_w(*W_CHUNKS[0])
            load_x(0, 0, 20)
            load_x(0, 20, NNODES)
            load_w(*W_CHUNKS[1])
            load_w(*W_CHUNKS[2])
            for rnd in (1, 2):
                load_x(rnd, 0, NNODES)

            def chain(n, wlo, xlo, bank, phalf, rnd):
                """Accumulation chain of node n, round rnd (chain-major:
                consecutive matmuls share the psum bank, new block each)."""
                nblk = len(src[n])
                ops = []
                for i in range(nblk):
                    bidx = off[n] + i
                    m = src[n][i]
                    c0 = rnd * XSEG + pos[m] * R
                    ops.append(dict(
                        out=bank[phalf:phalf + 64, :R],
                        lhsT=wt[wlo:wlo + 64, bidx * 64:(bidx + 1) * 64],
                        rhs=xt[xlo:xlo + 64, c0:c0 + R],
                        start=(i == 0),
                        stop=(i == nblk - 1),
                    ))
                return ops

            step = 0
            for rnd in range(NROUNDS):
                roc = rnd * RNDCOLS
                for j, (na, nb) in enumerate(pairs):
                    ab = pb[(2 * step) % 8]
                    cd = pb[(2 * step + 1) % 8]
                    step += 1
                    chains = [chain(na, 0, 0, ab, 0, rnd)]          # T0
                    if nb is not None:
                        chains.append(chain(nb, 0, 0, ab, 64, rnd))  # T2
                    chains.append(chain(na, 64, 64, cd, 0, rnd))    # T8
                    if nb is not None:
                        chains.append(chain(nb, 64, 64, cd, 64, rnd))  # T10
                    for i in range(max(len(c) for c in chains)):
                        for c in chains:
                            if i < len(c):
                                nc.tensor.matmul(**c[i])
                    # evacuate both banks into staging; alternate engines
                    rows = 128 if nb is not None else 64
                    sa = ot[0:rows, roc + 2 * j * R:roc + (2 * j + 1) * R]
                    sb = ot[0:rows, roc + (2 * j + 1) * R:roc + (2 * j + 2) * R]
                    if j % 2 == 0:
                        nc.vector.tensor_copy(sa, ab[0:rows, :R])
                        nc.scalar.copy(sb, cd[0:rows, :R])
                    else:
                        nc.scalar.copy(sa, ab[0:rows, :R])
                        nc.vector.tensor_copy(sb, cd[0:rows, :R])
                    if j in FLUSHES[rnd]:
                        fl = FLUSHES[rnd]
                        f0 = 0 if j == fl[0] else fl[fl.index(j) - 1] + 1
                        nc.sync.dma_start(
                            outd[:, roc + 2 * f0 * R:roc + 2 * (j + 1) * R],
                            ot[:, roc + 2 * f0 * R:roc + 2 * (j + 1) * R],
                        )

    nc.compile()
    _strip_redundant_ldweights(nc)
    return nc


def kernel(x, edge_index, W1, W2, W3, b1, b2, b3):
    from concourse.bass_utils import run_bass_kernel_spmd

    x = np.asarray(x, dtype=np.float32)
    edge_index = np.asarray(edge_index)
    Ws = [np.asarray(W, dtype=np.float64) for W in (W1, W2, W3)]
    bias = sum(np.asarray(b, dtype=np.float64) for b in (b1, b2, b3))

    plan = _plan(edge_index, Ws)
    key = (edge_index.tobytes(),)
    if _PROGRAM_CACHE.get("key") != key:
        _PROGRAM_CACHE["nc"] = _build_program(plan)
        _PROGRAM_CACHE["key"] = key
    nc = _PROGRAM_CACHE["nc"]

    # pack x per round segment: [c_in, node(first-use order), row] fp16
    # per partition half (slab A rows [0,1200) low, B [1200,2400) high);
    # 1/wscale (the fp8 weight scale) is folded in here (exact: pow2)
    x16 = (x * (1.0 / plan["wscale"])).astype(np.float16).reshape(
        N_CORES, ROWS_LOC, NNODES, C
    )
    fu = plan["fu"]
    xr = np.empty((N_CORES, 128, NROUNDS * XSEG), dtype=np.float16)
    for half in (0, 1):
        for rnd in range(NROUNDS):
            r0 = half * RSLAB + rnd * R
            blk = x16[:, r0:r0 + R]                 # [core, R, node, c]
            p = blk.transpose(0, 3, 2, 1)[:, :, fu, :]  # [core, c, fu, R]
            xr[:, half * 64:(half + 1) * 64, rnd * XSEG:(rnd + 1) * XSEG] = (
                p.reshape(N_CORES, C, XSEG)
            )

    w8 = plan["w8"]
    wdup = np.ascontiguousarray(np.concatenate([w8, w8], axis=0))
    in_maps = [{"xin": xr[i], "wdev": wdup} for i in range(N_CORES)]
    res = run_bass_kernel_spmd(nc, in_maps, list(range(N_CORES)), **_RUN_KW)
    _PROGRAM_CACHE["last_result"] = res

    # unpack: round block r, step j: A slot at 2j*R (T0 na low / T2 nb
    # high, slab A rows), B slot at (2j+1)*R (T8/T10, slab B rows)
    od = np.stack([res.results[i]["outd"] for i in range(N_CORES)])
    out = np.empty((N_CORES, ROWS_LOC, NNODES, C), dtype=np.float32)
    for rnd in range(NROUNDS):
        roc = rnd * RNDCOLS
        for j, (na, nb) in enumerate(plan["pairs"]):
            for side in (0, 1):                  # 0 = slab A, 1 = slab B
                r0 = side * RSLAB + rnd * R
                c0 = roc + (2 * j + side) * R
                piece = od[:, :, c0:c0 + R]
                out[:, r0:r0 + R, na, :] = (
                    piece[:, 0:64].transpose(0, 2, 1).astype(np.float32)
                )
                if nb is not None:
                    out[:, r0:r0 + R, nb, :] = (
                        piece[:, 64:128].transpose(0, 2, 1).astype(np.float32)
                    )
    out += bias.astype(np.float32)[None, None, None, :]
    return np.ascontiguousarray(out.reshape(B, T, NNODES, C))
